# revision 16
# baseline (speedup 1.0000x reference)
"""Trainium2 Bass kernel for DigitConvolutionalModel (conv3x3 -> 3-layer MLP).

Strategy (v3, bf16):
  - Pure data parallel over 8 NeuronCores: batch 65536 -> 8192 per core.
  - Host folds the 3x3 valid conv (28x28 -> 26x26) into W1:
        h1 = relu(conv(x) @ W1 + b1) = relu(x @ (C @ W1) + b1)
    where C (784, 676) is the sparse conv unfold matrix (exact, fp64).
  - All matmul operands in bf16 (error budget 2e-2; bf16 gives ~3e-3).
    PSUM accumulation fp32; biases fp32; final logits fp32.
  - M-tiles are 125/100 wide (NOT 128): a 128-wide bf16 stationary tile
    triggers the compiler's Fast Weight Load, which stops LDWEIGHTS from
    overlapping the running matmul (measured +26 ns/matmul).
  - Transposed layout: activations [feat_part, batch_free]; x pre-
    transposed on host to (784, 8192)/core, output (10, 8192) transposed
    back on host.
  - Pipeline per 512-column chunk: L1(c) | L2(c-1) | L3(c-2), bias+ReLU
    fused into ScalarE activation reading PSUM.
  - Engine/queue discipline: Scalar runs ONLY activations (plus the w1
    weight DMAs that finish before the first ACT); all x prefetch on the
    Sync HWDGE queue; small params + output stores on gpsimd SWDGE.
  - PE warmup matmuls bridge the DMA lead-in so the HAM clock-gate is
    warm (2.4 GHz) when real matmuls start.
"""

import os
import sys

sys.path.insert(0, "/opt/trn_rl_repo")

import numpy as np
import ml_dtypes

import concourse.bass as bass
import concourse.tile as tile
from concourse import mybir
import bass_rust
from concourse.bass_utils import run_bass_kernel_spmd

BF16_NP = np.dtype(ml_dtypes.bfloat16)

NCORES = 8
B = 65536
BC = B // NCORES          # 8192 rows per core
CHUNK = 512               # moving-dim tile (one PSUM bank of fp32)
NCHUNK = BC // CHUNK      # 16

NK1, K1T = 7, 112         # L1 contraction 784 = 7 x 112
M1, NM1, M1T = 500, 4, 125
NK2, K2T = 4, 125         # L2 contraction 500 = 4 x 125
M2, NM2, M2T = 200, 2, 100
NK3, K3T = 2, 100         # L3 contraction 200 = 2 x 100
M3 = 10

F32 = mybir.dt.float32
BF16 = mybir.dt.bfloat16

NWARM = int(os.environ.get("KERNEL_NWARM", "30"))


def _hoist_pe_waits(nc):
    """Move semaphore waits off PE MATMUL/LDWEIGHTS onto wait-only NOPs
    inserted before the preceding LDWEIGHTS. A wait carried by the MM (or
    its LDW) stalls the NX at that queue slot, breaking LDWEIGHTS
    pull-ahead (~96 ns/wait of PE dead time). On a NOP one slot earlier
    the same wait resolves while the previous matmul is still streaming.
    Safe: semaphores are monotone and engine order is preserved."""
    ctr = 0
    for f in nc.m.functions:
        for bb in f.blocks:
            insts = bb.instructions
            i = 0
            while i < len(insts):
                inst = insts[i]
                if not isinstance(inst, (mybir.InstMatmult, mybir.InstLdweights)):
                    i += 1
                    continue
                if "PE" not in str(inst.engine):
                    i += 1
                    continue
                si = inst.sync_info
                waits = list(si.on_wait) if (si and si.on_wait) else []
                if not waits:
                    i += 1
                    continue
                # insertion point: before the immediately preceding
                # LDWEIGHTS when this is its MATMUL, else before inst
                j = i
                if (isinstance(inst, mybir.InstMatmult) and i > 0
                        and isinstance(insts[i - 1], mybir.InstLdweights)):
                    j = i - 1
                inst.sync_info = bass_rust.SyncInfo(
                    on_wait=[], on_update=list(si.on_update or []))
                nop = mybir.InstNoOp(name=f"WHOIST-{ctr}", ins=[], outs=[])
                ctr += 1
                nop.engine = inst.engine
                nop.sync_info = bass_rust.SyncInfo(
                    on_wait=waits, on_update=[])
                insts.insert(j, nop)
                i += 2
    return ctr


def _split_excess_waits(nc, max_waits=1):
    """This walrus build caps sync-wait commands per instruction (Drain at 1).
    Hoist extra waits onto wait-only nops inserted just before, same engine."""
    ctr = 0
    for f in nc.m.functions:
        for bb in f.blocks:
            insts = bb.instructions
            i = 0
            while i < len(insts):
                inst = insts[i]
                si = inst.sync_info
                waits = list(si.on_wait) if (si and si.on_wait) else []
                if len(waits) > max_waits:
                    keep = waits[-max_waits:]
                    extra = waits[:-max_waits]
                    inst.sync_info = bass_rust.SyncInfo(
                        on_wait=keep, on_update=list(si.on_update or []))
                    nops = []
                    for j in range(0, len(extra), max_waits):
                        nop = mybir.InstNoOp(
                            name=f"WSPLIT-{ctr}", ins=[], outs=[])
                        ctr += 1
                        nop.engine = inst.engine
                        nop.sync_info = bass_rust.SyncInfo(
                            on_wait=extra[j:j + max_waits], on_update=[])
                        nops.append(nop)
                    insts[i:i] = nops
                    i += len(nops)
                i += 1
    return ctr


def build_bass():
    nc = bass.Bass(target_bir_lowering=False)
    Relu = mybir.ActivationFunctionType.Relu
    Ident = mybir.ActivationFunctionType.Identity

    xh = nc.declare_dram_parameter("xh", [NK1, K1T, BC], BF16, isOutput=False)
    w1 = nc.declare_dram_parameter("w1", [NK1, K1T, M1], BF16, isOutput=False)
    w2 = nc.declare_dram_parameter("w2", [NK2, K2T, M2], BF16, isOutput=False)
    w3 = nc.declare_dram_parameter("w3", [NK3, K3T, M3], BF16, isOutput=False)
    b1 = nc.declare_dram_parameter("b1", [M1T, NM1], F32, isOutput=False)
    b2 = nc.declare_dram_parameter("b2", [M2T, NM2], F32, isOutput=False)
    b3 = nc.declare_dram_parameter("b3", [M3, 1], F32, isOutput=False)
    out = nc.declare_dram_parameter("out", [M3, BC], F32, isOutput=True)

    with tile.TileContext(nc) as tc:
        with (
            tc.tile_pool(name="singles", bufs=1) as singles,
            tc.tile_pool(name="xg", bufs=1) as xg,
            tc.tile_pool(name="h1p", bufs=2) as h1p,
            tc.tile_pool(name="h2p", bufs=2) as h2p,
            tc.tile_pool(name="op", bufs=2) as op,
            tc.tile_pool(name="ps1p", bufs=4, space="PSUM") as ps1p,
            tc.tile_pool(name="ps2p", bufs=2, space="PSUM") as ps2p,
            tc.tile_pool(name="ps3p", bufs=1, space="PSUM") as ps3p,
            tc.tile_pool(name="pswp", bufs=1, space="PSUM") as pswp,
        ):
            # --- PE warmup: keep the HAM clock-gate fed while DMAs land.
            # Short N=64 matmuls (~55 ns each cold) bridge the ~2.5 us
            # until the first x/w tiles arrive without delaying real work.
            wt = singles.tile([1, CHUNK], BF16)
            nc.vector.memset(wt, 0.0)
            psw = pswp.tile([M1T, CHUNK], F32)
            for _ in range(NWARM):
                # 1-col stationary keeps the per-warmup LDWEIGHTS ~free
                nc.tensor.matmul(psw[0:1, 0:64], lhsT=wt[:, 0:1],
                                 rhs=wt[:, 0:64], start=True, stop=True)
            # --- weights/biases.
            # w1 per-k on the Scalar HWDGE queue: 7 descriptors, done
            # (~12us) before the first real ACT needs ScalarE (~15us).
            w1_ks = []
            for k in range(NK1):
                t = singles.tile([K1T, M1], BF16, tag=f"w1_{k}")
                nc.scalar.dma_start(out=t, in_=w1[k])
                w1_ks.append(t)
            # tiny dummy ACT (after the w1 descriptors!) so the one-time
            # ACT_TABLE_LOAD (~1.5us on ScalarE) happens during the DMA
            # lead-in, not before the first real activation.
            dumm = singles.tile([1, 1], F32)
            nc.scalar.activation(out=dumm, in_=psw[0:1, 0:1], func=Relu,
                                 bias=0.0, scale=1.0)
            # small params on gpsimd SWDGE (off the critical path)
            w2_s = singles.tile([K2T, NK2, M2], BF16)
            nc.gpsimd.dma_start(out=w2_s, in_=w2.rearrange("k p m -> p k m"))
            w3_s = singles.tile([K3T, NK3, M3], BF16)
            nc.gpsimd.dma_start(out=w3_s, in_=w3.rearrange("k p m -> p k m"))
            b1_s = singles.tile([M1T, NM1], F32)
            nc.gpsimd.dma_start(out=b1_s, in_=b1[:, :])
            b2_s = singles.tile([M2T, NM2], F32)
            nc.gpsimd.dma_start(out=b2_s, in_=b2[:, :])
            b3_s = singles.tile([M3, 1], F32)
            nc.gpsimd.dma_start(out=b3_s, in_=b3[:, :])

            # --- x prefetch, all on the Sync HWDGE queue (ScalarE must
            # stay free for ACTs). Per-chunk loads first for a fast PE
            # start, then progressively larger grouped loads.
            groups = [(0, 1), (1, 1), (2, 2), (4, 2), (6, 2), (8, 4), (12, 4)]
            chunk_src = {}
            for g0, glen in groups:
                xks = []
                for k in range(NK1):
                    t = xg.tile([K1T, glen * CHUNK], BF16, tag=f"x_{g0}_{k}")
                    nc.sync.dma_start(
                        out=t, in_=xh[k, :, g0 * CHUNK:(g0 + glen) * CHUNK])
                    xks.append(t)
                for cc in range(g0, g0 + glen):
                    chunk_src[cc] = (xks, (cc - g0) * CHUNK)

            h1_tiles = [None] * NCHUNK
            h2_tiles = [None] * NCHUNK
            for step in range(NCHUNK + 2):
                # Tensor order per step: L1(c), L2(c-1), L3(c-2).
                # Scalar order per step: h2-ACTs(c-1), out-ACT(c-2),
                # h1-ACTs(c) LAST — so the next step's first tensor wait
                # (on the h1 ACTs) subsumes every other cross-engine wait
                # and the scheduler can drop them (each surviving wait
                # costs ~96 ns of PE NX stall).
                # stage 1 matmuls: L1 for chunk c.
                # Chunk 0 runs k-major (4 matmuls per arriving k-tile, all
                # PSUM banks accumulating) so the PE keeps pace with the
                # one-descriptor-per-0.65us DMA startup stream; later
                # chunks run m-major.
                ps1s = None
                if step < NCHUNK:
                    c = step
                    xks, xoff = chunk_src[c]
                    ps1s = [ps1p.tile([M1T, CHUNK], F32, name="ps1")
                            for m in range(NM1)]
                    order = ([(m, k) for k in range(NK1) for m in range(NM1)]
                             if c == 0 else
                             [(m, k) for m in range(NM1) for k in range(NK1)])
                    for m, k in order:
                        nc.tensor.matmul(
                            ps1s[m],
                            lhsT=w1_ks[k][:, m * M1T:(m + 1) * M1T],
                            rhs=xks[k][:, xoff:xoff + CHUNK],
                            start=(k == 0), stop=(k == NK1 - 1))
                # stage 2: L2 matmuls for chunk c-1, then its h2 ACTs
                if 1 <= step <= NCHUNK:
                    c = step - 1
                    h1s = h1_tiles[c]
                    ps2s = []
                    for m in range(NM2):
                        ps2 = ps2p.tile([M2T, CHUNK], F32, name="ps2")
                        for k in range(NK2):
                            nc.tensor.matmul(
                                ps2,
                                lhsT=w2_s[:, k, m * M2T:(m + 1) * M2T],
                                rhs=h1s[k],
                                start=(k == 0), stop=(k == NK2 - 1))
                        ps2s.append(ps2)
                    h2s = []
                    for m in range(NM2):
                        h2 = h2p.tile([M2T, CHUNK], BF16, tag=f"h2_{m}")
                        nc.scalar.activation(
                            out=h2, in_=ps2s[m], func=Relu,
                            bias=b2_s[:, m:m + 1], scale=1.0)
                        h2s.append(h2)
                    h2_tiles[c] = h2s
                # stage 3: L3 for chunk c-2 + out ACT + store
                if step >= 2:
                    c = step - 2
                    h2s = h2_tiles[c]
                    ps3 = ps3p.tile([M3, CHUNK], F32)
                    for k in range(NK3):
                        nc.tensor.matmul(
                            ps3, lhsT=w3_s[:, k, :], rhs=h2s[k],
                            start=(k == 0), stop=(k == NK3 - 1))
                    o_t = op.tile([M3, CHUNK], F32)
                    nc.scalar.activation(
                        out=o_t, in_=ps3, func=Ident,
                        bias=b3_s[:, 0:1], scale=1.0)
                    nc.gpsimd.dma_start(
                        out=out[:, c * CHUNK:(c + 1) * CHUNK], in_=o_t)
                # stage 1 ACTs last in the scalar stream
                if ps1s is not None:
                    c = step
                    h1s = []
                    for m in range(NM1):
                        h1 = h1p.tile([M1T, CHUNK], BF16, tag=f"h1_{m}")
                        nc.scalar.activation(
                            out=h1, in_=ps1s[m], func=Relu,
                            bias=b1_s[:, m:m + 1], scale=1.0)
                        h1s.append(h1)
                    h1_tiles[c] = h1s

    _hoist_pe_waits(nc)
    _split_excess_waits(nc)
    return nc


_NC_CACHE = None


def _get_nc():
    global _NC_CACHE
    if _NC_CACHE is None:
        _NC_CACHE = build_bass()
    return _NC_CACHE


def _conv_unfold(conv_w):
    """C (784, 676): x_flat @ C == flatten(valid 3x3 xcorr of x as 28x28)."""
    C = np.zeros((784, 676), dtype=np.float64)
    w = np.asarray(conv_w, dtype=np.float64)
    for i in range(26):
        for j in range(26):
            q = 26 * i + j
            for di in range(3):
                for dj in range(3):
                    C[28 * (i + di) + (j + dj), q] += w[di, dj]
    return C


def kernel(x, conv_w, W1, b1, W2, b2, W3, b3, _trace=False, _tmpdir=None):
    x = np.asarray(x, dtype=np.float32)
    conv_w = np.asarray(conv_w, dtype=np.float32)
    W1 = np.asarray(W1, dtype=np.float32)
    b1 = np.asarray(b1, dtype=np.float32)
    W2 = np.asarray(W2, dtype=np.float32)
    b2 = np.asarray(b2, dtype=np.float32)
    W3 = np.asarray(W3, dtype=np.float32)
    b3 = np.asarray(b3, dtype=np.float32)

    C = _conv_unfold(conv_w)
    W1f = (C @ W1.astype(np.float64)).astype(np.float32)  # (784, 500)

    w1_h = W1f.reshape(NK1, K1T, M1).astype(BF16_NP)
    w2_h = W2.reshape(NK2, K2T, M2).astype(BF16_NP)
    w3_h = W3.reshape(NK3, K3T, M3).astype(BF16_NP)
    b1_h = np.ascontiguousarray(b1.reshape(NM1, M1T).T)
    b2_h = np.ascontiguousarray(b2.reshape(NM2, M2T).T)
    b3_h = np.ascontiguousarray(b3.reshape(M3, 1))

    nc = _get_nc()
    in_maps = []
    for c in range(NCORES):
        xT = x[c * BC:(c + 1) * BC, :].T  # (784, 8192)
        xh = np.ascontiguousarray(xT).astype(BF16_NP).reshape(NK1, K1T, BC)
        in_maps.append({
            "xh": xh, "w1": w1_h, "b1": b1_h,
            "w2": w2_h, "b2": b2_h, "w3": w3_h, "b3": b3_h,
        })

    res = run_bass_kernel_spmd(
        nc, in_maps, list(range(NCORES)), trace=_trace, tmpdir=_tmpdir)
    out = np.empty((B, M3), dtype=np.float32)
    for c in range(NCORES):
        out[c * BC:(c + 1) * BC, :] = res.results[c]["out"].T
    if _trace:
        return out, res
    return out
